# revision 18
# baseline (speedup 1.0000x reference)
"""Instruction-minimal Trainium2 Bass kernel for nn_GCN_15590731285230 (v12).

Rig model (measured, see mb*.py + ab.py interleaved A/B): matmuls with a
loaded stationary are ~2us (hw-decoded); a stationary change ~54us;
non-PE instructions execute near-serially and PSUM-reading drains are the
dominant cost (~190-550us each, NOT hidden by double-buffering; gpsimd
cannot touch PSUM, DMA cannot read PSUM). Wide (>512 col) matmuls are
rejected by walrus, so 64 (scores) + 64 (Z) matmuls is the hard floor.
The v3 structure (big 8-bank drains, 16-deep interleaved PSUM
accumulation for Z) measured as a local optimum over 16x[128,2048]
drains, SBUF accumulator chains, per-round engine-split drains, and
hardware loops (all slower or a wash end-to-end in interleaved A/B).
v12 = v3 plus the measured micro-wins:

  * xT loaded directly via a transposing DMA; xnat as "(t p) f -> p t f"
    so node enumeration is natural (stall[p, t, n] = S[n, 128t+p]).
  * DMA QUEUE SPREADING (the biggest single win, ~8-12% together): the
    three loads issue from three different DMA-capable queues (wsb on
    Act, xnat on gpsimd, xT gather on sync); the mid-kernel r0 bounce
    issues from gpsimd; the final store from Act.
  * yT in one 4-bank PSUM tensor: 4 matmuls + 1 drain.
  * pass A: 64 matmuls, 8 drains of [128, 4096], drain engine ALTERNATES
    Act/DVE between rounds (independent queues, ~5% end-to-end win).
  * softmax: strided max reduce + gpsimd all-reduce + sub + Schraudolph
    exp on DVE (Act Exp is ~5.3ms here) with the f32->i32 convert FOLDED
    into the bias-add's output dtype (one fewer [128,32768] op).
  * r0 DRAM-bounce DMAs issued right after the exp so the bounce
    overlaps the Z pass.
  * Z^T: 64 matmuls accumulating in 4 banks; normalization folded into
    the single drain (tensor_mul by broadcast 1/s).
  * h1^T = relu(w1^T Znorm + x^T): 4 matmuls + add + relu.
  * tail (row 0 only): partition-broadcast r0, mul + reduce (a fused
    tensor_tensor_reduce hard-crashes this rig), 1 matmul with w2, then
    one scalar_tensor_tensor fuses the 1/s0 scale + h1 row-0 residual.

Per batch b (core b):
    R  = softmax(x wr x^T, axis=-1);  h1 = relu(R x w1 + x)
    out_b = relu(R[0,:] @ h1 @ w2 + h1[0,:])
"""

import sys

if "/opt/trn_rl_repo" not in sys.path:
    sys.path.insert(0, "/opt/trn_rl_repo")

from contextlib import ExitStack

import numpy as np

import concourse.bacc as bacc
import concourse.bass as bass
import concourse.bass_isa as bass_isa
import concourse.mybir as mybir
import concourse.tile as tile
from concourse.bass_utils import run_bass_kernel_spmd

P = 128
D = 128
B = 8
F32 = mybir.dt.float32
AF = mybir.ActivationFunctionType
AX = mybir.AxisListType
ALU = mybir.AluOpType
ROP = bass_isa.ReduceOp


def _bcast_free(ap, count):
    """Insert a stride-0 dim of size `count` after the partition dim."""
    return bass.AP(tensor=ap.tensor, offset=ap.offset,
                   ap=[list(ap.ap[0]), [0, count]] + [list(d) for d in ap.ap[1:]])


def _squeeze(ap):
    """Drop unit free dims (keep dim 0) so DMA AP balancing stays <= 3 dims."""
    return bass.AP(tensor=ap.tensor, offset=ap.offset,
                   ap=[list(ap.ap[0])] +
                      [list(d) for d in ap.ap[1:] if d[1] > 1])


def build_kernel(n=2048, repeat=1, dbg_stage=None):
    nt = n // P              # m chunks
    w5 = min(512, n)         # matmul moving-operand width
    nc = bacc.Bacc()
    x_d = nc.dram_tensor("x", [n, D], F32, kind="ExternalInput")
    wall_d = nc.dram_tensor("wall", [3 * D, D], F32, kind="ExternalInput")
    out_d = nc.dram_tensor("out", [1, D], F32, kind="ExternalOutput")
    r0_d = nc.dram_tensor("r0scratch", [1, n], F32, kind="Internal")
    dbg_d = (nc.dram_tensor("dbg", [P, n], F32, kind="ExternalOutput")
             if dbg_stage else None)

    with tile.TileContext(nc) as tc, ExitStack() as ctx:
        sg = ctx.enter_context(tc.tile_pool(name="sg", bufs=1))
        scr = ctx.enter_context(tc.tile_pool(name="scr", bufs=1))
        bb = ctx.enter_context(tc.tile_pool(name="bb", bufs=1))
        st = ctx.enter_context(tc.tile_pool(name="st", bufs=1))

        for _rep in range(repeat):
            # one DMA for all three weights: wsb[p, k, :] = w_k[p, :]
            # loads spread across three DMA-capable queues so their setup
            # times overlap (sync keeps the critical xT gather)
            wsb = sg.tile([P, 3, P], F32, tag="wsb")
            nc.scalar.dma_start(wsb, wall_d[:].rearrange("(k p) f -> p k f",
                                                         p=P))
            wr_sb = wsb[:, 0, :]
            w1_sb = wsb[:, 1, :]
            w2_sb = wsb[:, 2, :]

            # natural-enumeration chunked x: xnat[p, t, f] = x[128 t + p, f]
            xnat = sg.tile([P, nt, P], F32, tag="xnat")
            nc.gpsimd.dma_start(xnat, x_d[:].rearrange("(t p) f -> p t f", p=P))
            # transposed x via DMA gather: xT[f, m] = x[m, f]
            xT = sg.tile([P, n], F32, tag="xT")
            nc.sync.dma_start(xT, x_d[:].rearrange("n f -> f n"))

            L = {None: 99, "xT": 1, "passA": 2, "softmax": 3,
                 "h1": 4, "r0": 5, "ttr": 6, "o2": 7, "fin": 8}[dbg_stage]
            if L == 1:
                nc.sync.dma_start(dbg_d[:], xT)
                continue

            # yT = (x wr)^T : yT[g, n] = sum_f wr[f, g] xT[f, n]
            yT = sg.tile([P, n], F32, tag="yT")
            with tc.tile_pool(name="psy", bufs=1, space="PSUM") as psy:
                yp = psy.tile([P, n], F32, tag="yp")
                for j in range(0, n, w5):
                    nc.tensor.matmul(yp[:, j:j + w5], lhsT=wr_sb,
                                     rhs=xT[:, j:j + w5],
                                     start=True, stop=True)
                nc.vector.tensor_copy(yT, yp)

            # ---- pass A: ST[m, n] = S[n, m] with m = 128 t + p ----
            stall = sg.tile([P, nt, n], F32, tag="stall")
            stall_flat = stall.rearrange("p t n -> p (t n)")
            with tc.tile_pool(name="psA", bufs=1, space="PSUM") as psA:
                for g in range(0, nt, 2):
                    sp = psA.tile([P, 2 * n], F32, tag="sp")
                    for k in range(2):
                        for j in range(0, n, w5):
                            nc.tensor.matmul(
                                sp[:, k * n + j:k * n + j + w5],
                                lhsT=xT[:, (g + k) * P:(g + k + 1) * P],
                                rhs=yT[:, j:j + w5],
                                start=True, stop=True)
                    # alternate the drain engine so consecutive rounds'
                    # drains land on independent queues
                    if (g // 2) % 2 == 0:
                        nc.scalar.copy(stall_flat[:, g * n:(g + 2) * n], sp)
                    else:
                        nc.vector.tensor_copy(
                            stall_flat[:, g * n:(g + 2) * n], sp)

            if L == 2:
                nc.sync.dma_start(dbg_d[:], stall[:, 0, :])
                continue

            # ---- softmax over m (partitions x chunks), per column n ----
            mx_pt = scr.tile([P, n], F32, tag="scr")
            nc.vector.tensor_reduce(mx_pt, stall.rearrange("p t n -> p n t"),
                                    axis=AX.X, op=ALU.max)
            mtile = bb.tile([P, n], F32, tag="bb")
            nc.gpsimd.partition_all_reduce(mtile, mx_pt, channels=P,
                                           reduce_op=ROP.max)
            nc.vector.tensor_sub(stall_flat, stall_flat,
                                 _bcast_free(mtile[:], nt))
            # exp via Schraudolph bit trick on DVE (Act-engine Exp costs
            # ~5.3 ms for this tensor on this rig; this chain ~0.1 ms):
            #   bits = int32(A * max(u, -87) + B);  E = bitcast_fp32(bits)
            # A = 2^23/ln2, B = 127*2^23 - 486411 (RMS-optimal bias,
            # ~1.5% rms weight error; softmax normalization absorbs most).
            nc.vector.tensor_scalar(
                stall_flat, stall_flat, -87.0,
                scalar2=12102203.161561485,
                op0=ALU.max, op1=ALU.mult)
            # the f32->i32 convert is folded into the add's output dtype
            nc.vector.tensor_scalar_add(stall_flat.bitcast(mybir.dt.int32),
                                        stall_flat, 1064866805.0)
            # r0 bounce hoisted here: runs on DMA queues, overlaps the Z
            # pass. r0 (unnormalized) = exp'd scores at n=0: stall[p, t, 0]
            # written to DRAM at natural position m = 128 t + p, reloaded
            # flat. (A direct SBUF gather to a [1, n] row needs a 4-dim DMA
            # AP, which is rejected.)
            # issued from the gpsimd DMA queue to keep the sync queue free
            nc.gpsimd.dma_start(
                r0_d[:].rearrange("o (t p) -> p t o", p=P),
                stall[:, :, 0:1])
            r0row = scr.tile([1, n], F32, tag="r0row")
            nc.gpsimd.dma_start(r0row, r0_d[:])
            etsum = scr.tile([P, n], F32, tag="scr")
            nc.vector.tensor_reduce(etsum, stall.rearrange("p t n -> p n t"),
                                    axis=AX.X, op=ALU.add)
            stile = bb.tile([P, n], F32, tag="bb")   # reuses mtile slot
            nc.gpsimd.partition_all_reduce(stile, etsum, channels=P,
                                           reduce_op=ROP.add)
            nc.vector.reciprocal(stile, stile)       # 1/s, all partitions

            if L == 3:
                nc.sync.dma_start(dbg_d[:], stile)
                continue

            # ---- Z^T accumulation over chunks; h1T = relu(w1^T Zn + xT) ----
            h1t = sg.tile([P, n], F32, tag="h1t")
            with tc.tile_pool(name="psB", bufs=1, space="PSUM") as psB:
                ztp = psB.tile([P, n], F32, tag="zt")
                for t in range(nt):
                    for j in range(0, n, w5):
                        nc.tensor.matmul(ztp[:, j:j + w5],
                                         lhsT=xnat[:, t, :],
                                         rhs=stall[:, t, j:j + w5],
                                         start=(t == 0), stop=(t == nt - 1))
                # znorm = ZT * (1/s): drains PSUM and normalizes in one op
                znorm = sg.tile([P, n], F32, tag="yT")   # reuses yT slot
                nc.vector.tensor_mul(znorm, ztp, stile)

                hp = psB.tile([P, n], F32, tag="zt")  # reuses ztp banks
                for j in range(0, n, w5):
                    nc.tensor.matmul(hp[:, j:j + w5], lhsT=w1_sb,
                                     rhs=znorm[:, j:j + w5],
                                     start=True, stop=True)
                nc.vector.tensor_add(h1t, hp, xT)
                nc.vector.tensor_relu(h1t, h1t)
                if L == 4:
                    nc.sync.dma_start(dbg_d[:], h1t)
                    continue

                # ---- tail: out = relu(r0 @ h1 @ w2 + h1[0, :]) ----
                # (r0 bounce DMAs were issued before the Z pass)
                r0tile = sg.tile([P, n], F32, tag="xnat")  # reuses xnat slot
                nc.gpsimd.partition_broadcast(r0tile, r0row)
                if L == 5:
                    nc.sync.dma_start(dbg_d[:], r0tile)
                    continue
                wsum = sg.tile([P, n], F32, tag="yT")      # reuses znorm slot
                v = st.tile([P, 1], F32, tag="v")
                # (tensor_tensor_reduce hard-crashes the exec unit on this
                # rig - NRT_EXEC_UNIT_UNRECOVERABLE - so mul + reduce.)
                nc.vector.tensor_mul(wsum, h1t, r0tile)
                nc.vector.tensor_reduce(v, wsum, axis=AX.X, op=ALU.add)
                if L == 6:
                    nc.sync.dma_start(dbg_d[:], wsum)
                    nc.sync.dma_start(dbg_d[:, n - 1:n], v)
                    continue
                # o2 = w2^T v as a [128, 1] partition column, so the h1 row-0
                # residual (h1t[:, 0:1], same orientation) adds directly.
                o2 = psB.tile([P, 1], F32, tag="zt")  # reuses hp banks
                nc.tensor.matmul(o2, lhsT=w2_sb, rhs=v, start=True, stop=True)
                # fin = (o2 * 1/s0) + h1[0,:] fused into one op; then relu
                fin = st.tile([P, 1], F32, tag="fin")
                nc.vector.scalar_tensor_tensor(fin, o2, stile[:, 0:1],
                                               h1t[:, 0:1],
                                               op0=ALU.mult, op1=ALU.add)
                nc.vector.tensor_scalar_max(fin, fin, 0.0)
                if L == 8:
                    nc.sync.dma_start(dbg_d[:, 0:1], fin)
                    continue
                # DRAM side carries the transpose: SBUF APs must keep the
                # partition dim first (moving it to a free dim reads garbage).
                # Issued from the Act queue (sync queue handles the loads).
                nc.scalar.dma_start(out_d[:].rearrange("o p -> p o"), fin)

    nc.compile()
    return nc


_CACHE = {}


def kernel(x, w1, w2, wr):
    x = np.ascontiguousarray(np.asarray(x), dtype=np.float32)
    w1 = np.ascontiguousarray(np.asarray(w1), dtype=np.float32)
    w2 = np.ascontiguousarray(np.asarray(w2), dtype=np.float32)
    wr = np.ascontiguousarray(np.asarray(wr), dtype=np.float32)
    b, n, d = x.shape
    if "nc" not in _CACHE:
        _CACHE["nc"] = build_kernel(n)
    nc = _CACHE["nc"]
    wall = np.ascontiguousarray(np.concatenate([wr, w1, w2], axis=0))
    in_maps = [{"x": x[i], "wall": wall} for i in range(b)]
    res = run_bass_kernel_spmd(nc, in_maps, core_ids=list(range(b)))
    return np.stack([res.results[i]["out"][0] for i in range(b)])

